# revision 2
# baseline (speedup 1.0000x reference)
"""ChamferDistance kernel for 8 Trainium2 NeuronCores.

Problem: B=2 batches, N=M=16384 points, 3-D. For both directions
(xyz1->xyz2, xyz2->xyz1) find per-point nearest-neighbor squared
distance + argmin index. Outputs (dist1, dist2, idx1, idx2).

Sharding: 4 (batch, direction) groups x 2 query-halves = 8 cores.
Each core: 8192 queries vs 16384 refs, fully independent.

Per 128-query row:
  PE (fp32r): d' = 2a.b - |a|^2 - |b|^2 into PSUM (negated sq dist,
      so argmax d' == argmin dist; fp32r has ~11-bit mantissa => used
      for candidate SELECTION only).
  DVE: per-32 chunk maxima -> top-8 chunks (InstMax/InstMaxIndex).
  DMA gather of those 8 chunks' points from a DRAM table.
  DVE: exact-fp32 recompute of the 256 candidate distances with the
      reference's formula/rounding ((asq+bsq) - 2*(a.b)), exact min +
      first-occurrence argmin (ties -> smallest global index).
"""
import sys

if "/opt/trn_rl_repo" not in sys.path:
    sys.path.insert(0, "/opt/trn_rl_repo")

import numpy as np

B = 2
N = 16384        # points per cloud (queries == refs count)
NQPC = 8192      # queries per core (half a cloud)
NROWS = NQPC // 128   # 64 rows of 128 queries
M = N            # refs per core
CH = 32          # chunk size for candidate selection
NCHUNK = M // CH      # 512
NCAND = 8        # top chunks refined exactly
SUP = 2048       # PSUM super-chunk (4 banks)
NSUP = M // SUP  # 8

_CACHE = {}


def _build():
    import concourse.mybir as mybir
    import concourse.tile as tile
    from concourse import bacc

    f32 = mybir.dt.float32
    f32r = mybir.dt.float32r
    u32 = mybir.dt.uint32
    i16 = mybir.dt.int16
    A = mybir.AluOpType
    X = mybir.AxisListType.X
    XY = mybir.AxisListType.XY

    nc = bacc.Bacc()
    qr_comb = nc.declare_dram_parameter("qr_comb", [5, NQPC + M], f32r, isOutput=False)
    pscal = nc.declare_dram_parameter("pscal", [128, NROWS * 4 + CH], f32, isOutput=False)
    ctab = nc.declare_dram_parameter("ctab", [NCHUNK, 4 * CH], f32, isOutput=False)
    out_dist = nc.declare_dram_parameter("out_dist", [NQPC], f32, isOutput=True)
    out_idx = nc.declare_dram_parameter("out_idx", [NQPC], u32, isOutput=True)

    with tile.TileContext(nc) as tc:
        with (
            tc.tile_pool(name="const", bufs=1) as cpool,
            tc.tile_pool(name="work", bufs=2) as wp,
            tc.tile_pool(name="ps", bufs=2, space="PSUM") as ps,
            tc.tile_pool(name="dr", bufs=2, space="DRAM") as dr,
        ):
            qr_sb = cpool.tile([5, NQPC + M], f32r)
            ps_sb = cpool.tile([128, NROWS * 4 + CH], f32)
            nc.sync.dma_start(out=qr_sb[:], in_=qr_comb[:])
            nc.sync.dma_start(out=ps_sb[:], in_=pscal[:])
            dist_acc = cpool.tile([128, NROWS], f32)
            idx_acc = cpool.tile([128, NROWS], u32)
            iota_b = ps_sb[:, NROWS * 4:NROWS * 4 + CH]

            for r in range(NROWS):
                lhsT = qr_sb[:, r * 128:(r + 1) * 128]
                cmax = wp.tile([128, NCHUNK], f32, tag="cmax")
                for s in range(NSUP):
                    psum = ps.tile([128, SUP // CH, CH], f32, tag="psum")
                    for j in range(SUP // 512):
                        nc.tensor.matmul(
                            out=psum[:, j * 16:(j + 1) * 16, :],
                            lhsT=lhsT,
                            rhs=qr_sb[:, NQPC + s * SUP + j * 512:NQPC + s * SUP + (j + 1) * 512],
                            start=True, stop=True)
                    nc.vector.tensor_reduce(
                        cmax[:, s * (SUP // CH):(s + 1) * (SUP // CH)], psum[:], X, A.max)

                v8 = wp.tile([128, 8], f32, tag="v8")
                c8 = wp.tile([128, 8], u32, tag="c8")
                nc.vector.max(v8[:], cmax[:])
                nc.vector.max_index(c8[:], v8[:], cmax[:])

                # wrapped-16 int16 index layout for dma_gather, via DRAM round-trip
                c16 = wp.tile([128, NCAND], i16, tag="c16")
                nc.vector.tensor_copy(out=c16[:], in_=c8[:, 0:NCAND])
                idxd = dr.tile([16, 8 * NCAND], i16, tag="idxd")
                nc.sync.dma_start(
                    out=idxd[:].rearrange("q (j g) -> g q j", j=NCAND, g=8), in_=c16[:])
                wrapped = wp.tile([128, 8 * NCAND], i16, tag="wrapped")
                nc.sync.dma_start(
                    out=wrapped[:], in_=idxd[:].unsqueeze(0).broadcast_to([8, 16, 8 * NCAND]))

                cand = wp.tile([128, NCAND, 4 * CH], f32, tag="cand")
                nc.gpsimd.dma_gather(
                    out_ap=cand[:], in_ap=ctab[:], idxs_ap=wrapped[:],
                    num_idxs=128 * NCAND, num_idxs_reg=128 * NCAND, elem_size=4 * CH)

                # exact-fp32 recompute, matching the reference's rounding:
                # d = (asq + bsq) - 2*(ax*bx + ay*by + az*bz), sequential fp32
                bx = cand[:, :, 0 * CH:1 * CH]
                by = cand[:, :, 1 * CH:2 * CH]
                bz = cand[:, :, 2 * CH:3 * CH]
                bs = cand[:, :, 3 * CH:4 * CH]
                r4 = r * 4
                t = wp.tile([128, NCAND, CH], f32, tag="t")
                nc.vector.tensor_scalar(
                    out=t[:], in0=bx, scalar1=ps_sb[:, r4:r4 + 1], scalar2=None, op0=A.mult)
                nc.vector.scalar_tensor_tensor(
                    out=t[:], in0=by, scalar=ps_sb[:, r4 + 1:r4 + 2], in1=t[:],
                    op0=A.mult, op1=A.add)
                nc.vector.scalar_tensor_tensor(
                    out=t[:], in0=bz, scalar=ps_sb[:, r4 + 2:r4 + 3], in1=t[:],
                    op0=A.mult, op1=A.add)
                u = wp.tile([128, NCAND, CH], f32, tag="u")
                nc.vector.tensor_scalar(
                    out=u[:], in0=bs, scalar1=ps_sb[:, r4 + 3:r4 + 4], scalar2=None, op0=A.add)
                d = wp.tile([128, NCAND, CH], f32, tag="d")
                nc.vector.tensor_tensor(out=d[:], in0=u[:], in1=t[:], op=A.subtract)

                # min + first-occurrence argmin by global index
                nc.vector.tensor_reduce(dist_acc[:, r:r + 1], d[:], XY, A.min)
                cf = wp.tile([128, NCAND], f32, tag="cf")
                nc.vector.tensor_copy(out=cf[:], in_=c8[:, 0:NCAND])
                gidx = wp.tile([128, NCAND, CH], f32, tag="gidx")
                nc.vector.scalar_tensor_tensor(
                    out=gidx[:], in0=cf[:, :, None].broadcast_to([128, NCAND, CH]),
                    scalar=float(CH), in1=iota_b[:, None, :].broadcast_to([128, NCAND, CH]),
                    op0=A.mult, op1=A.add)
                pen = wp.tile([128, NCAND, CH], f32, tag="pen")
                nc.vector.tensor_scalar(
                    out=pen[:], in0=d[:], scalar1=dist_acc[:, r:r + 1], scalar2=-32768.0,
                    op0=A.is_le, op1=A.mult)
                nc.vector.tensor_tensor(out=pen[:], in0=pen[:], in1=gidx[:], op=A.add)
                selt = wp.tile([128, 1], f32, tag="selt")
                nc.vector.tensor_reduce(selt[:], pen[:], XY, A.min)
                idxf = wp.tile([128, 1], f32, tag="idxf")
                nc.vector.tensor_scalar(
                    out=idxf[:], in0=selt[:], scalar1=32768.0, scalar2=None, op0=A.add)
                nc.vector.tensor_copy(out=idx_acc[:, r:r + 1], in_=idxf[:])

            nc.sync.dma_start(
                out=out_dist[:].rearrange("(r p) -> p r", r=NROWS, p=128), in_=dist_acc[:])
            nc.sync.dma_start(
                out=out_idx[:].rearrange("(r p) -> p r", r=NROWS, p=128), in_=idx_acc[:])
    nc.finalize()
    return nc


def _get_nc():
    if "nc" not in _CACHE:
        _CACHE["nc"] = _build()
    return _CACHE["nc"]


def _prep_core(q, refs):
    """q: [NQPC,3] queries, refs: [M,3]. Returns the in_map for one core."""
    qx, qy, qz = q[:, 0], q[:, 1], q[:, 2]
    rx, ry, rz = refs[:, 0], refs[:, 1], refs[:, 2]
    asq = ((qx * qx + qy * qy) + qz * qz).astype(np.float32)
    bsq = ((rx * rx + ry * ry) + rz * rz).astype(np.float32)
    qaug = np.stack([2 * qx, 2 * qy, 2 * qz, -asq,
                     -np.ones(NQPC, np.float32)]).astype(np.float32)
    raug = np.stack([rx, ry, rz, np.ones(M, np.float32), bsq]).astype(np.float32)
    qr_comb = np.concatenate([qaug, raug], axis=1)

    qs = np.stack([2 * qx, 2 * qy, 2 * qz, asq], axis=1).astype(np.float32)  # [NQPC,4]
    qs = qs.reshape(NROWS, 128, 4).transpose(1, 0, 2).reshape(128, NROWS * 4)
    iota = np.broadcast_to(np.arange(CH, dtype=np.float32), (128, CH))
    pscal = np.concatenate([qs, iota], axis=1).astype(np.float32)

    ct = refs.reshape(NCHUNK, CH, 3).transpose(0, 2, 1)           # [NCHUNK,3,CH]
    ctab = np.concatenate([ct, bsq.reshape(NCHUNK, 1, CH)], axis=1)
    ctab = np.ascontiguousarray(ctab.reshape(NCHUNK, 4 * CH), dtype=np.float32)
    return dict(qr_comb=np.ascontiguousarray(qr_comb),
                pscal=np.ascontiguousarray(pscal), ctab=ctab)


def _sample_check(dist1, dist2, xyz1, xyz2, n_samp=32):
    """Exact-f64 spot check; detects silent device corruption."""
    rng = np.random.default_rng(12345)
    bad = 0
    for batch in range(B):
        for direction in range(2):
            q = (xyz1, xyz2)[direction][batch]
            refs = (xyz2, xyz1)[direction][batch]
            d_out = (dist1, dist2)[direction][batch]
            qi = rng.integers(0, N, n_samp)
            dq = ((q[qi, None, :].astype(np.float64) -
                   refs[None, :, :].astype(np.float64)) ** 2).sum(-1)
            dmin = dq.min(axis=1)
            bad += int((np.abs(d_out[qi] - dmin) > 1e-4 * np.maximum(1.0, dmin)).sum())
    return bad <= 2


def kernel(xyz1: np.ndarray, xyz2: np.ndarray):
    import time as _time

    from concourse.bass_utils import run_bass_kernel_spmd

    xyz1 = np.asarray(xyz1, dtype=np.float32)
    xyz2 = np.asarray(xyz2, dtype=np.float32)

    in_maps = []
    for c in range(8):
        g, h = divmod(c, 2)
        batch, direction = divmod(g, 2)
        if direction == 0:
            q_full, refs = xyz1[batch], xyz2[batch]
        else:
            q_full, refs = xyz2[batch], xyz1[batch]
        q = q_full[h * NQPC:(h + 1) * NQPC]
        in_maps.append(_prep_core(q, refs))

    nc = _get_nc()
    out = None
    for attempt in range(3):
        try:
            res = run_bass_kernel_spmd(nc, in_maps, list(range(8)))
        except Exception:
            if attempt == 2:
                raise
            _time.sleep(45)
            continue
        _CACHE["last_results"] = res

        dist1 = np.zeros((B, N), np.float32)
        dist2 = np.zeros((B, N), np.float32)
        idx1 = np.zeros((B, N), np.int32)
        idx2 = np.zeros((B, N), np.int32)
        for c in range(8):
            g, h = divmod(c, 2)
            batch, direction = divmod(g, 2)
            r = res.results[c]
            dst = slice(h * NQPC, (h + 1) * NQPC)
            if direction == 0:
                dist1[batch, dst] = r["out_dist"]
                idx1[batch, dst] = r["out_idx"].astype(np.int32)
            else:
                dist2[batch, dst] = r["out_dist"]
                idx2[batch, dst] = r["out_idx"].astype(np.int32)
        out = (dist1, dist2, idx1, idx2)
        if _sample_check(dist1, dist2, xyz1, xyz2):
            return out
        _time.sleep(30)
    return out
